# revision 7
# baseline (speedup 1.0000x reference)
"""Trainium2 Bass kernel for CrossAttention (B=4, S=S_ctx=2048, D=1024, H=16, Hd=64).

Sharding: 8 cores = batch (4) x head-group (2 groups of 8 heads).
Each core computes, for its (b, g):
    q = x_b @ qw_g          (per-head mean-centering folded into qw on host)
    k = ctx_b @ kw_g        (same)
    v = ctx_b @ vw_g
    per-head LN (rstd only; mean is zero by construction), RoPE on q,
    softmax(q k^T / 8) v per head, partial out-proj with this group's proj_w
    rows.  Host sums the two group partials per batch and adds proj bias.

On-core dataflow (all f32):
  - per 128-token tile: DMA in, 8 PE-transposes -> channel-major tile,
    stationary for the K/V/Q projection matmuls (no full transposed copy)
  - LN-rstd (+ optional affine) and RoPE applied token-major (free-dim math),
    then Q/K PE-transposed into head-dim-major QT/KT for attention
  - logitsT[t, s] per head-pair via two row-tiled matmuls (K=64 each)
  - exp on ScalarE straight out of PSUM (1/sqrt(Hd) folded into activation scale)
  - out^T accumulated with stationary [V_h | ones]; ones row gives softmax denom
  - normalize via DVE reciprocal + DMA partition-broadcast
  - out-proj: attn-outT tiles stationary, proj_w moving, token-major result
"""

import numpy as np
from contextlib import ExitStack

import concourse.bacc as bacc
import concourse.bass as bass
import concourse.tile as tile
from concourse import mybir
from concourse.bass_utils import run_bass_kernel_spmd
from concourse.masks import make_identity

F32 = mybir.dt.float32
AF = mybir.ActivationFunctionType

B, S, DIM = 4, 2048, 1024
H, HD = 16, 64
G = 2                  # head groups (tensor-parallel dim)
HL = H // G            # heads per core = 8
DL = HL * HD           # local head dims = 512
P = 128
NT = S // P            # 16 token tiles
NR = DIM // P          # 8 channel tiles
EPS = 1e-5

_program_cache = {}
LAST_RUN = None        # BassKernelResults of most recent run (for test harness)


def _mk_ap(ap, dims):
    """Raw AP on the same tensor/offset with explicit [step, count] dims."""
    return bass.AP(tensor=ap.tensor, offset=ap.offset, ap=list(dims))


def _build_program(ln_affine_q, ln_affine_k, trace=False):
    nc = bacc.Bacc(None, target_bir_lowering=False, debug=False)

    x_d = nc.dram_tensor("x", [S, DIM], F32, kind="ExternalInput")
    ctx_d = nc.dram_tensor("ctx", [S, DIM], F32, kind="ExternalInput")
    qw_d = nc.dram_tensor("qw", [DIM, DL], F32, kind="ExternalInput")
    kw_d = nc.dram_tensor("kw", [DIM, DL], F32, kind="ExternalInput")
    vw_d = nc.dram_tensor("vw", [DIM, DL], F32, kind="ExternalInput")
    pw_d = nc.dram_tensor("pw", [DL, DIM], F32, kind="ExternalInput")
    cos_d = nc.dram_tensor("cos", [S, HD // 2], F32, kind="ExternalInput")
    sin_d = nc.dram_tensor("sin", [S, HD // 2], F32, kind="ExternalInput")
    qs_d = nc.dram_tensor("qs", [HD], F32, kind="ExternalInput")
    qb_d = nc.dram_tensor("qb", [HD], F32, kind="ExternalInput")
    ks_d = nc.dram_tensor("ks", [HD], F32, kind="ExternalInput")
    kb_d = nc.dram_tensor("kb", [HD], F32, kind="ExternalInput")
    y_d = nc.dram_tensor("y", [S, DIM], F32, kind="ExternalOutput")
    den_d = nc.dram_tensor("den_scratch", [4, HL, S // 4], F32)  # internal

    with tile.TileContext(nc) as tc, ExitStack() as top:
        const = top.enter_context(tc.tile_pool(name="const", bufs=1))
        ident = const.tile([P, P], F32)
        make_identity(nc, ident[:])

        eps_sb = const.tile([P, 1], F32)
        nc.vector.memset(eps_sb[:], EPS)

        cos_sb = const.tile([P, NT, HD // 2], F32)
        sin_sb = const.tile([P, NT, HD // 2], F32)
        nc.sync.dma_start(cos_sb[:], cos_d[:].rearrange("(i p) f -> p i f", p=P))
        nc.sync.dma_start(sin_sb[:], sin_d[:].rearrange("(i p) f -> p i f", p=P))

        ln_tiles = {}
        for flag, s_t, b_t, key in (
            (ln_affine_q, qs_d, qb_d, "q"),
            (ln_affine_k, ks_d, kb_d, "k"),
        ):
            if flag:
                st = const.tile([P, HD], F32)
                bt = const.tile([P, HD], F32)
                nc.gpsimd.dma_start(st[:], s_t[:].partition_broadcast(P))
                nc.gpsimd.dma_start(bt[:], b_t[:].partition_broadcast(P))
                ln_tiles[key] = (st, bt)

        persist = top.enter_context(tc.tile_pool(name="persist", bufs=1))
        QT = persist.tile([P, HL // 2, S], F32)         # [d-par, pair, s]
        KT = persist.tile([P, HL // 2, S], F32)         # [d-par, pair, t]
        Vaug = persist.tile([P, NT, HL, HD + 1], F32)   # [t-par, t-tile, h, e|1]
        rstdQ = persist.tile([P, NT, HL], F32)
        rstdK = persist.tile([P, NT, HL], F32)
        nc.vector.memset(Vaug[:, :, :, HD : HD + 1], 1.0)

        def rstd_of(work, nat, dst):
            """dst[:, :] = 1/sqrt(mean(nat^2 per head) + eps); nat is [P, DL]."""
            sq = work.tile([P, DL], F32, tag="sq")
            nc.vector.tensor_mul(sq[:], nat[:], nat[:])
            sums = work.tile([P, HL], F32, tag="sums")
            nc.vector.tensor_reduce(
                sums[:],
                sq[:].rearrange("p (h d) -> p h d", h=HL),
                axis=mybir.AxisListType.X,
                op=mybir.AluOpType.add,
            )
            sdt = work.tile([P, HL], F32, tag="sdt")
            nc.scalar.activation(
                sdt[:], sums[:], AF.Sqrt, bias=eps_sb[:], scale=1.0 / HD
            )
            nc.vector.reciprocal(dst, sdt[:])

        def apply_affine(nat3, key):
            if key in ln_tiles:
                st, bt = ln_tiles[key]
                stb = _mk_ap(st[:], [st[:].ap[0], [0, HL], [1, HD]])
                btb = _mk_ap(bt[:], [bt[:].ap[0], [0, HL], [1, HD]])
                nc.vector.tensor_mul(nat3, nat3, stb)
                nc.vector.tensor_add(nat3, nat3, btb)

        # ================= projection phases =================
        with ExitStack() as ph:
            tp = ph.enter_context(tc.tile_pool(name="tin", bufs=3))
            xtp = ph.enter_context(tc.tile_pool(name="xt", bufs=2))
            wp = ph.enter_context(tc.tile_pool(name="w", bufs=1))
            work = ph.enter_context(tc.tile_pool(name="work", bufs=2))
            tr_ps = ph.enter_context(tc.tile_pool(name="tr_ps", bufs=2, space="PSUM"))
            mm_ps = ph.enter_context(tc.tile_pool(name="mm_ps", bufs=2, space="PSUM"))
            ot_ps = ph.enter_context(tc.tile_pool(name="ot_ps", bufs=2, space="PSUM"))

            kw_sb = wp.tile([P, NR, DL], F32)
            vw_sb = wp.tile([P, NR, DL], F32)
            qw_sb = wp.tile([P, NR, DL], F32)
            nc.sync.dma_start(kw_sb[:], kw_d[:].rearrange("(r p) d -> p r d", p=P))
            nc.sync.dma_start(vw_sb[:], vw_d[:].rearrange("(r p) d -> p r d", p=P))
            nc.sync.dma_start(qw_sb[:], qw_d[:].rearrange("(r p) d -> p r d", p=P))

            def transposed_tile(inp_dram, t):
                """Load token-tile t and return channel-major [P, NR, P] tile."""
                t_in = tp.tile([P, DIM], F32, tag="t_in")
                nc.sync.dma_start(t_in[:], inp_dram[t * P : (t + 1) * P, :])
                xt = xtp.tile([P, NR, P], F32, tag="xt")
                for r2 in range(2):
                    ps4 = tr_ps.tile([P, 4 * P], F32, tag="ps4")
                    for j in range(4):
                        nc.tensor.transpose(
                            ps4[:, j * P : (j + 1) * P],
                            t_in[:, (r2 * 4 + j) * P : (r2 * 4 + j + 1) * P],
                            ident[:],
                        )
                    nc.scalar.copy(xt[:, r2 * 4 : (r2 + 1) * 4, :], ps4[:])
                return xt

            def out_transpose(nat, OT, t):
                """PE-transpose token-major [P, DL] into OT[:, :, t*P:...]."""
                for r4 in range(DL // P):
                    psT = ot_ps.tile([P, P], F32, tag="psT")
                    nc.tensor.transpose(
                        psT[:], nat[:, r4 * P : (r4 + 1) * P], ident[:]
                    )
                    nc.scalar.copy(OT[:, r4, t * P : (t + 1) * P], psT[:])

            # context side: K and V
            for t in range(NT):
                xt = transposed_tile(ctx_d, t)
                ps_k = mm_ps.tile([P, DL], F32, tag="ps_k")
                ps_v = mm_ps.tile([P, DL], F32, tag="ps_v")
                for r in range(NR):
                    nc.tensor.matmul(
                        ps_k[:], xt[:, r, :], kw_sb[:, r, :],
                        start=(r == 0), stop=(r == NR - 1),
                    )
                for r in range(NR):
                    nc.tensor.matmul(
                        ps_v[:], xt[:, r, :], vw_sb[:, r, :],
                        start=(r == 0), stop=(r == NR - 1),
                    )
                nc.scalar.copy(
                    Vaug[:, t, :, 0:HD],
                    ps_v[:].rearrange("p (h d) -> p h d", h=HL),
                )
                k_nat = work.tile([P, DL], F32, tag="k_nat")
                nc.scalar.copy(k_nat[:], ps_k[:])
                rstd_of(work, k_nat, rstdK[:, t, :])
                rb = _mk_ap(rstdK[:, t, :], [rstdK[:].ap[0], [1, HL], [0, HD]])
                k3 = k_nat[:].rearrange("p (h d) -> p h d", h=HL)
                nc.vector.tensor_mul(k3, k3, rb)
                apply_affine(k3, "k")
                out_transpose(k_nat, KT, t)

            # x side: Q (+ RoPE)
            for t in range(NT):
                xt = transposed_tile(x_d, t)
                ps_q = mm_ps.tile([P, DL], F32, tag="ps_k")
                for r in range(NR):
                    nc.tensor.matmul(
                        ps_q[:], xt[:, r, :], qw_sb[:, r, :],
                        start=(r == 0), stop=(r == NR - 1),
                    )
                q_nat = work.tile([P, DL], F32, tag="k_nat")
                nc.scalar.copy(q_nat[:], ps_q[:])
                rstd_of(work, q_nat, rstdQ[:, t, :])
                rb = _mk_ap(rstdQ[:, t, :], [rstdQ[:].ap[0], [1, HL], [0, HD]])
                q3 = q_nat[:].rearrange("p (h d) -> p h d", h=HL)
                nc.vector.tensor_mul(q3, q3, rb)
                apply_affine(q3, "q")
                # RoPE: view [p, h, 2, 32]
                qcos = work.tile([P, DL], F32, tag="qcos")
                qsin = work.tile([P, DL], F32, tag="qsin")
                cb = _mk_ap(cos_sb[:, t, :],
                            [cos_sb[:].ap[0], [0, HL], [0, 2], [1, HD // 2]])
                sb = _mk_ap(sin_sb[:, t, :],
                            [sin_sb[:].ap[0], [0, HL], [0, 2], [1, HD // 2]])
                q4 = q_nat[:].rearrange("p (h two f) -> p h two f", h=HL, two=2)
                qcos4 = qcos[:].rearrange("p (h two f) -> p h two f", h=HL, two=2)
                qsin4 = qsin[:].rearrange("p (h two f) -> p h two f", h=HL, two=2)
                nc.vector.tensor_mul(qcos4, q4, cb)
                nc.vector.tensor_mul(qsin4, q4, sb)
                nc.vector.tensor_sub(
                    q4[:, :, 0, :], qcos4[:, :, 0, :], qsin4[:, :, 1, :]
                )
                nc.vector.tensor_add(
                    q4[:, :, 1, :], qsin4[:, :, 0, :], qcos4[:, :, 1, :]
                )
                out_transpose(q_nat, QT, t)

        # ================= attention + out-proj =================
        with ExitStack() as ph:
            pwp = ph.enter_context(tc.tile_pool(name="pw", bufs=1))
            l_ps = ph.enter_context(tc.tile_pool(name="l_ps", bufs=2, space="PSUM"))
            o_ps = ph.enter_context(tc.tile_pool(name="o_ps", bufs=1, space="PSUM"))
            y_ps = ph.enter_context(tc.tile_pool(name="y_ps", bufs=2, space="PSUM"))
            ex_pool = ph.enter_context(tc.tile_pool(name="ex", bufs=3))
            ao_pool = ph.enter_context(tc.tile_pool(name="ao", bufs=2))
            nrm_pool = ph.enter_context(tc.tile_pool(name="nrm", bufs=4))
            y_pool = ph.enter_context(tc.tile_pool(name="y", bufs=3))

            pw_sb = pwp.tile([P, DL // P, DIM], F32)
            nc.sync.dma_start(pw_sb[:], pw_d[:].rearrange("(r p) n -> p r n", p=P))

            NSC = 4
            SC = S // NSC          # 512

            for sc in range(NSC):
                aoT = ao_pool.tile([P, DL // P, SC], F32, tag="aoT")
                for r in range(HL // 2):
                    ps_o = o_ps.tile([HD + 1, 2, SC], F32, tag="ps_o")
                    for t in range(NT):
                        ps_l = l_ps.tile([P, 2 * SC], F32, tag="ps_l")
                        nc.tensor.matmul(
                            ps_l[:, 0:SC],
                            KT[0:HD, r, t * P : (t + 1) * P],
                            QT[0:HD, r, sc * SC : (sc + 1) * SC],
                            start=True, stop=True,
                            tile_position=(0, 0),
                        )
                        nc.tensor.matmul(
                            ps_l[:, SC : 2 * SC],
                            KT[HD:P, r, t * P : (t + 1) * P],
                            QT[HD:P, r, sc * SC : (sc + 1) * SC],
                            start=True, stop=True,
                            tile_position=(HD, 0),
                        )
                        ex = ex_pool.tile([P, 2 * SC], F32, tag="ex")
                        nc.scalar.activation(
                            ex[:], ps_l[:], AF.Exp, scale=1.0 / np.sqrt(HD)
                        )
                        for j in range(2):
                            nc.tensor.matmul(
                                ps_o[:, j, :],
                                Vaug[:, t, 2 * r + j, :],
                                ex[:, j * SC : (j + 1) * SC],
                                start=(t == 0), stop=(t == NT - 1),
                            )
                    for j in range(2):
                        den = nrm_pool.tile([P, SC], F32, tag="den")
                        nc.vector.reciprocal(
                            den[HD : HD + 1, :], ps_o[HD : HD + 1, j, :]
                        )
                        h = 2 * r + j
                        nc.sync.dma_start(den_d[sc, h, :], den[HD : HD + 1, :])
                        denB = nrm_pool.tile([HD, SC], F32, tag="denB")
                        nc.gpsimd.dma_start(
                            denB[:], den_d[sc, h, :].partition_broadcast(HD)
                        )
                        if j == 0:
                            nc.vector.tensor_mul(
                                aoT[0:HD, r, :], ps_o[0:HD, 0, :], denB[:]
                            )
                        else:
                            tmpB = nrm_pool.tile([HD, SC], F32, tag="tmpB")
                            nc.vector.tensor_mul(tmpB[:], ps_o[0:HD, 1, :], denB[:])
                            nc.gpsimd.dma_start(aoT[HD:P, r, :], tmpB[:])
                for si in range(SC // P):
                    y_sb = y_pool.tile([P, DIM], F32, tag="y_sb")
                    for n2 in range(2):
                        ps_y = y_ps.tile([P, DIM // 2], F32, tag="ps_y")
                        for r in range(DL // P):
                            nc.tensor.matmul(
                                ps_y[:],
                                aoT[:, r, si * P : (si + 1) * P],
                                pw_sb[:, r, n2 * (DIM // 2) : (n2 + 1) * (DIM // 2)],
                                start=(r == 0), stop=(r == DL // P - 1),
                            )
                        nc.vector.tensor_copy(
                            y_sb[:, n2 * (DIM // 2) : (n2 + 1) * (DIM // 2)], ps_y[:]
                        )
                    row0 = sc * SC + si * P
                    nc.sync.dma_start(y_d[row0 : row0 + P, :], y_sb[:])

    nc.compile()
    return nc


def _center_mat():
    m = np.eye(HD, dtype=np.float64) - np.ones((HD, HD), dtype=np.float64) / HD
    return np.kron(np.eye(H, dtype=np.float64), m)  # [DIM, DIM] block-diag


def kernel(x, context, q_w, kv_w, qn_scale, qn_bias, kn_scale, kn_bias,
           proj_w, proj_b, _trace=False):
    global LAST_RUN
    x = np.asarray(x, np.float32)
    context = np.asarray(context, np.float32)
    q_w = np.asarray(q_w, np.float32)
    kv_w = np.asarray(kv_w, np.float32)
    proj_w = np.asarray(proj_w, np.float32)
    proj_b = np.asarray(proj_b, np.float32)
    qn_scale = np.asarray(qn_scale, np.float32)
    qn_bias = np.asarray(qn_bias, np.float32)
    kn_scale = np.asarray(kn_scale, np.float32)
    kn_bias = np.asarray(kn_bias, np.float32)

    ln_affine_q = not (np.all(qn_scale == 1.0) and np.all(qn_bias == 0.0))
    ln_affine_k = not (np.all(kn_scale == 1.0) and np.all(kn_bias == 0.0))

    key = (ln_affine_q, ln_affine_k)
    if key not in _program_cache:
        _program_cache[key] = _build_program(*key)
    nc = _program_cache[key]

    C = _center_mat()
    qw_c = (q_w.astype(np.float64) @ C).astype(np.float32)
    kw_c = (kv_w[:, :DIM].astype(np.float64) @ C).astype(np.float32)
    vw_full = np.ascontiguousarray(kv_w[:, DIM:])

    inv_freq = 1.0 / (10000.0 ** (np.arange(0, HD, 2, dtype=np.float32) / HD))
    ang = np.arange(S, dtype=np.float32)[:, None] * inv_freq
    cos_t = np.cos(ang).astype(np.float32)
    sin_t = np.sin(ang).astype(np.float32)

    in_maps = []
    for core in range(B * G):
        b, g = divmod(core, G)
        sl = slice(g * DL, (g + 1) * DL)
        in_maps.append({
            "x": np.ascontiguousarray(x[b]),
            "ctx": np.ascontiguousarray(context[b]),
            "qw": np.ascontiguousarray(qw_c[:, sl]),
            "kw": np.ascontiguousarray(kw_c[:, sl]),
            "vw": np.ascontiguousarray(vw_full[:, sl]),
            "pw": np.ascontiguousarray(proj_w[sl, :]),
            "cos": cos_t, "sin": sin_t,
            "qs": qn_scale, "qb": qn_bias, "ks": kn_scale, "kb": kn_bias,
        })

    LAST_RUN = run_bass_kernel_spmd(
        nc, in_maps, list(range(B * G)), trace=_trace
    )
    res = LAST_RUN.results

    out = np.zeros((B, S, DIM), np.float32)
    for core in range(B * G):
        out[core // G] += res[core]["y"]
    out += proj_b[None, None, :]
    return out


# revision 11
# speedup vs baseline: 1.0183x; 1.0183x over previous
"""Trainium2 Bass kernel for CrossAttention (B=4, S=S_ctx=2048, D=1024, H=16, Hd=64).

Sharding: 8 cores = batch (4) x head-group (2 groups of 8 heads).
Each core computes, for its (b, g):
    q = x_b @ qw_g          (per-head mean-centering folded into qw on host)
    k = ctx_b @ kw_g        (same)
    v = ctx_b @ vw_g
    per-head LN (rstd only; mean is zero by construction), RoPE on q,
    softmax(q k^T / 8) v per head, partial out-proj with this group's proj_w
    rows.  Host sums the two group partials per batch and adds proj bias.

On-core dataflow (all f32):
  - per 128-token tile: DMA in, 8 PE-transposes -> channel-major tile,
    stationary for the K/V/Q projection matmuls (no full transposed copy)
  - LN-rstd (+ optional affine) and RoPE applied token-major (free-dim math),
    then Q/K PE-transposed into head-dim-major QT/KT for attention
  - logitsT[t, s] per head-pair via two row-tiled matmuls (K=64 each)
  - exp on ScalarE straight out of PSUM (1/sqrt(Hd) folded into activation scale)
  - out^T accumulated with stationary [V_h | ones]; ones row gives softmax denom
  - normalize via DVE reciprocal + DMA partition-broadcast
  - out-proj: attn-outT tiles stationary, proj_w moving, token-major result
"""

import numpy as np
from contextlib import ExitStack

import concourse.bacc as bacc
import concourse.bass as bass
import concourse.tile as tile
from concourse import mybir
from concourse.bass_utils import run_bass_kernel_spmd
from concourse.masks import make_identity

F32 = mybir.dt.float32
AF = mybir.ActivationFunctionType

B, S, DIM = 4, 2048, 1024
H, HD = 16, 64
G = 2                  # head groups (tensor-parallel dim)
HL = H // G            # heads per core = 8
DL = HL * HD           # local head dims = 512
P = 128
NT = S // P            # 16 token tiles
NR = DIM // P          # 8 channel tiles
EPS = 1e-5

_program_cache = {}
LAST_RUN = None        # BassKernelResults of most recent run (for test harness)


def _mk_ap(ap, dims):
    """Raw AP on the same tensor/offset with explicit [step, count] dims."""
    return bass.AP(tensor=ap.tensor, offset=ap.offset, ap=list(dims))


def _build_program(ln_affine_q, ln_affine_k, trace=False):
    nc = bacc.Bacc(None, target_bir_lowering=False, debug=False)

    x_d = nc.dram_tensor("x", [S, DIM], F32, kind="ExternalInput")
    ctx_d = nc.dram_tensor("ctx", [S, DIM], F32, kind="ExternalInput")
    qw_d = nc.dram_tensor("qw", [DIM, DL], F32, kind="ExternalInput")
    kw_d = nc.dram_tensor("kw", [DIM, DL], F32, kind="ExternalInput")
    vw_d = nc.dram_tensor("vw", [DIM, DL], F32, kind="ExternalInput")
    pw_d = nc.dram_tensor("pw", [DL, DIM], F32, kind="ExternalInput")
    cos_d = nc.dram_tensor("cos", [S, HD // 2], F32, kind="ExternalInput")
    sin_d = nc.dram_tensor("sin", [S, HD // 2], F32, kind="ExternalInput")
    qs_d = nc.dram_tensor("qs", [HD], F32, kind="ExternalInput")
    qb_d = nc.dram_tensor("qb", [HD], F32, kind="ExternalInput")
    ks_d = nc.dram_tensor("ks", [HD], F32, kind="ExternalInput")
    kb_d = nc.dram_tensor("kb", [HD], F32, kind="ExternalInput")
    y_d = nc.dram_tensor("y", [S, DIM], F32, kind="ExternalOutput")
    den_d = nc.dram_tensor("den_scratch", [4, HL, S // 4], F32)  # internal

    with tile.TileContext(nc) as tc, ExitStack() as top:
        const = top.enter_context(tc.tile_pool(name="const", bufs=1))
        ident = const.tile([P, P], F32)
        make_identity(nc, ident[:])

        eps_sb = const.tile([P, 1], F32)
        nc.vector.memset(eps_sb[:], EPS)
        ones_sb = const.tile([P, HD], F32)
        nc.vector.memset(ones_sb[:], 1.0)

        cos_sb = const.tile([P, NT, HD // 2], F32)
        sin_sb = const.tile([P, NT, HD // 2], F32)
        nc.sync.dma_start(cos_sb[:], cos_d[:].rearrange("(i p) f -> p i f", p=P))
        nc.sync.dma_start(sin_sb[:], sin_d[:].rearrange("(i p) f -> p i f", p=P))

        ln_tiles = {}
        for flag, s_t, b_t, key in (
            (ln_affine_q, qs_d, qb_d, "q"),
            (ln_affine_k, ks_d, kb_d, "k"),
        ):
            if flag:
                st = const.tile([P, HD], F32)
                bt = const.tile([P, HD], F32)
                nc.gpsimd.dma_start(st[:], s_t[:].partition_broadcast(P))
                nc.gpsimd.dma_start(bt[:], b_t[:].partition_broadcast(P))
                ln_tiles[key] = (st, bt)

        persist = top.enter_context(tc.tile_pool(name="persist", bufs=1))
        QT = persist.tile([P, HL // 2, S], F32)         # [d-par, pair, s]
        KT = persist.tile([P, HL // 2, S], F32)         # [d-par, pair, t]
        Vaug = persist.tile([P, NT, HL, HD + 1], F32)   # [t-par, t-tile, h, e|1]
        rstdQ = persist.tile([P, NT, HL], F32)
        rstdK = persist.tile([P, NT, HL], F32)
        nc.vector.memset(Vaug[:, :, :, HD : HD + 1], 1.0)

        def rstd_of(work, nat, dst):
            """dst[:, :] = 1/sqrt(mean(nat^2 per head) + eps); nat is [P, DL]."""
            sq = work.tile([P, DL], F32, tag="sq")
            nc.vector.tensor_mul(sq[:], nat[:], nat[:])
            sums = work.tile([P, HL], F32, tag="sums")
            nc.vector.tensor_reduce(
                sums[:],
                sq[:].rearrange("p (h d) -> p h d", h=HL),
                axis=mybir.AxisListType.X,
                op=mybir.AluOpType.add,
            )
            sdt = work.tile([P, HL], F32, tag="sdt")
            nc.scalar.activation(
                sdt[:], sums[:], AF.Sqrt, bias=eps_sb[:], scale=1.0 / HD
            )
            nc.vector.reciprocal(dst, sdt[:])

        def apply_affine(nat3, key):
            if key in ln_tiles:
                st, bt = ln_tiles[key]
                stb = _mk_ap(st[:], [st[:].ap[0], [0, HL], [1, HD]])
                btb = _mk_ap(bt[:], [bt[:].ap[0], [0, HL], [1, HD]])
                nc.vector.tensor_mul(nat3, nat3, stb)
                nc.vector.tensor_add(nat3, nat3, btb)

        # ================= projection phases =================
        with ExitStack() as ph:
            tp = ph.enter_context(tc.tile_pool(name="tin", bufs=3))
            xtp = ph.enter_context(tc.tile_pool(name="xt", bufs=2))
            wp = ph.enter_context(tc.tile_pool(name="w", bufs=1))
            work = ph.enter_context(tc.tile_pool(name="work", bufs=2))
            tr_ps = ph.enter_context(tc.tile_pool(name="tr_ps", bufs=2, space="PSUM"))
            mm_ps = ph.enter_context(tc.tile_pool(name="mm_ps", bufs=2, space="PSUM"))
            ot_ps = ph.enter_context(tc.tile_pool(name="ot_ps", bufs=2, space="PSUM"))

            kw_sb = wp.tile([P, NR, DL], F32)
            vw_sb = wp.tile([P, NR, DL], F32)
            qw_sb = wp.tile([P, NR, DL], F32)
            nc.sync.dma_start(kw_sb[:], kw_d[:].rearrange("(r p) d -> p r d", p=P))
            nc.sync.dma_start(vw_sb[:], vw_d[:].rearrange("(r p) d -> p r d", p=P))
            nc.sync.dma_start(qw_sb[:], qw_d[:].rearrange("(r p) d -> p r d", p=P))

            def transposed_tile(inp_dram, t):
                """Load token-tile t and return channel-major [P, NR, P] tile."""
                t_in = tp.tile([P, DIM], F32, tag="t_in")
                nc.sync.dma_start(t_in[:], inp_dram[t * P : (t + 1) * P, :])
                xt = xtp.tile([P, NR, P], F32, tag="xt")
                for r2 in range(2):
                    ps4 = tr_ps.tile([P, 4 * P], F32, tag="ps4")
                    for j in range(4):
                        nc.tensor.transpose(
                            ps4[:, j * P : (j + 1) * P],
                            t_in[:, (r2 * 4 + j) * P : (r2 * 4 + j + 1) * P],
                            ident[:],
                        )
                    nc.scalar.copy(xt[:, r2 * 4 : (r2 + 1) * 4, :], ps4[:])
                return xt

            def out_transpose(nat, OT, t):
                """PE-transpose token-major [P, DL] into OT[:, :, t*P:...]."""
                for r4 in range(DL // P):
                    psT = ot_ps.tile([P, P], F32, tag="psT")
                    nc.tensor.transpose(
                        psT[:], nat[:, r4 * P : (r4 + 1) * P], ident[:]
                    )
                    nc.scalar.copy(OT[:, r4, t * P : (t + 1) * P], psT[:])

            # context side: K and V
            for t in range(NT):
                xt = transposed_tile(ctx_d, t)
                ps_k = mm_ps.tile([P, DL], F32, tag="ps_k")
                ps_v = mm_ps.tile([P, DL], F32, tag="ps_v")
                for r in range(NR):
                    nc.tensor.matmul(
                        ps_k[:], xt[:, r, :], kw_sb[:, r, :],
                        start=(r == 0), stop=(r == NR - 1),
                    )
                for r in range(NR):
                    nc.tensor.matmul(
                        ps_v[:], xt[:, r, :], vw_sb[:, r, :],
                        start=(r == 0), stop=(r == NR - 1),
                    )
                nc.scalar.copy(
                    Vaug[:, t, :, 0:HD],
                    ps_v[:].rearrange("p (h d) -> p h d", h=HL),
                )
                k_nat = work.tile([P, DL], F32, tag="k_nat")
                nc.scalar.copy(k_nat[:], ps_k[:])
                rstd_of(work, k_nat, rstdK[:, t, :])
                rb = _mk_ap(rstdK[:, t, :], [rstdK[:].ap[0], [1, HL], [0, HD]])
                k3 = k_nat[:].rearrange("p (h d) -> p h d", h=HL)
                nc.vector.tensor_mul(k3, k3, rb)
                apply_affine(k3, "k")
                out_transpose(k_nat, KT, t)

            # x side: Q (+ RoPE)
            for t in range(NT):
                xt = transposed_tile(x_d, t)
                ps_q = mm_ps.tile([P, DL], F32, tag="ps_k")
                for r in range(NR):
                    nc.tensor.matmul(
                        ps_q[:], xt[:, r, :], qw_sb[:, r, :],
                        start=(r == 0), stop=(r == NR - 1),
                    )
                q_nat = work.tile([P, DL], F32, tag="k_nat")
                nc.scalar.copy(q_nat[:], ps_q[:])
                rstd_of(work, q_nat, rstdQ[:, t, :])
                rb = _mk_ap(rstdQ[:, t, :], [rstdQ[:].ap[0], [1, HL], [0, HD]])
                q3 = q_nat[:].rearrange("p (h d) -> p h d", h=HL)
                nc.vector.tensor_mul(q3, q3, rb)
                apply_affine(q3, "q")
                # RoPE: view [p, h, 2, 32]
                qcos = work.tile([P, DL], F32, tag="qcos")
                qsin = work.tile([P, DL], F32, tag="qsin")
                cb = _mk_ap(cos_sb[:, t, :],
                            [cos_sb[:].ap[0], [0, HL], [0, 2], [1, HD // 2]])
                sb = _mk_ap(sin_sb[:, t, :],
                            [sin_sb[:].ap[0], [0, HL], [0, 2], [1, HD // 2]])
                q4 = q_nat[:].rearrange("p (h two f) -> p h two f", h=HL, two=2)
                qcos4 = qcos[:].rearrange("p (h two f) -> p h two f", h=HL, two=2)
                qsin4 = qsin[:].rearrange("p (h two f) -> p h two f", h=HL, two=2)
                nc.vector.tensor_mul(qcos4, q4, cb)
                nc.vector.tensor_mul(qsin4, q4, sb)
                nc.vector.tensor_sub(
                    q4[:, :, 0, :], qcos4[:, :, 0, :], qsin4[:, :, 1, :]
                )
                nc.vector.tensor_add(
                    q4[:, :, 1, :], qsin4[:, :, 0, :], qcos4[:, :, 1, :]
                )
                out_transpose(q_nat, QT, t)

        # ================= attention + out-proj =================
        with ExitStack() as ph:
            pwp = ph.enter_context(tc.tile_pool(name="pw", bufs=1))
            l_ps = ph.enter_context(tc.tile_pool(name="l_ps", bufs=2, space="PSUM"))
            o_ps = ph.enter_context(tc.tile_pool(name="o_ps", bufs=2, space="PSUM"))
            ex_pool = ph.enter_context(tc.tile_pool(name="ex", bufs=3))
            ao_pool = ph.enter_context(tc.tile_pool(name="ao", bufs=2))
            nrm_pool = ph.enter_context(tc.tile_pool(name="nrm", bufs=4))
            y_pool = ph.enter_context(tc.tile_pool(name="y", bufs=3))

            pw_sb = pwp.tile([P, DL // P, DIM], F32)
            nc.sync.dma_start(pw_sb[:], pw_d[:].rearrange("(r p) n -> p r n", p=P))

            NSC = 4
            SC = S // NSC          # 512

            for sc in range(NSC):
                aoT = ao_pool.tile([P, DL // P, SC], F32, tag="aoT")
                for r in range(HL // 2):
                    ps_o = o_ps.tile([HD + 1, 2, SC], F32, tag="ps_o")
                    for t in range(NT):
                        ps_l = l_ps.tile([P, 2 * SC], F32, tag="ps_l")
                        nc.tensor.matmul(
                            ps_l[:, 0:SC],
                            KT[0:HD, r, t * P : (t + 1) * P],
                            QT[0:HD, r, sc * SC : (sc + 1) * SC],
                            start=True, stop=True,
                            tile_position=(0, 0),
                        )
                        nc.tensor.matmul(
                            ps_l[:, SC : 2 * SC],
                            KT[HD:P, r, t * P : (t + 1) * P],
                            QT[HD:P, r, sc * SC : (sc + 1) * SC],
                            start=True, stop=True,
                            tile_position=(HD, 0),
                        )
                        ex = ex_pool.tile([P, 2 * SC], F32, tag="ex")
                        nc.scalar.activation(
                            ex[:], ps_l[:], AF.Exp, scale=1.0 / np.sqrt(HD)
                        )
                        for j in range(2):
                            nc.tensor.matmul(
                                ps_o[:, j, :],
                                Vaug[:, t, 2 * r + j, :],
                                ex[:, j * SC : (j + 1) * SC],
                                start=(t == 0), stop=(t == NT - 1),
                            )
                    for j in range(2):
                        den = nrm_pool.tile([P, SC], F32, tag="den")
                        nc.vector.reciprocal(
                            den[HD : HD + 1, :], ps_o[HD : HD + 1, j, :]
                        )
                        # broadcast row 64 -> 64 partitions via K=1 matmul
                        ps_b = l_ps.tile([HD, SC], F32, tag="ps_l")
                        nc.tensor.matmul(
                            ps_b[:],
                            ones_sb[HD : HD + 1, :],
                            den[HD : HD + 1, :],
                            start=True, stop=True,
                            tile_position=(HD, 0),
                        )
                        denB = nrm_pool.tile([HD, SC], F32, tag="denB")
                        nc.vector.tensor_copy(denB[:], ps_b[:])
                        if j == 0:
                            nc.vector.tensor_mul(
                                aoT[0:HD, r, :], ps_o[0:HD, 0, :], denB[:]
                            )
                        else:
                            tmpB = nrm_pool.tile([HD, SC], F32, tag="tmpB")
                            nc.vector.tensor_mul(tmpB[:], ps_o[0:HD, 1, :], denB[:])
                            nc.gpsimd.dma_start(aoT[HD:P, r, :], tmpB[:])
                for si in range(SC // P):
                    y_sb = y_pool.tile([P, DIM], F32, tag="y_sb")
                    for n2 in range(2):
                        ps_y = o_ps.tile([P, DIM // 2], F32, tag="ps_o")
                        for r in range(DL // P):
                            nc.tensor.matmul(
                                ps_y[:],
                                aoT[:, r, si * P : (si + 1) * P],
                                pw_sb[:, r, n2 * (DIM // 2) : (n2 + 1) * (DIM // 2)],
                                start=(r == 0), stop=(r == DL // P - 1),
                            )
                        nc.vector.tensor_copy(
                            y_sb[:, n2 * (DIM // 2) : (n2 + 1) * (DIM // 2)], ps_y[:]
                        )
                    row0 = sc * SC + si * P
                    nc.sync.dma_start(y_d[row0 : row0 + P, :], y_sb[:])

    nc.compile()
    return nc


def _center_mat():
    m = np.eye(HD, dtype=np.float64) - np.ones((HD, HD), dtype=np.float64) / HD
    return np.kron(np.eye(H, dtype=np.float64), m)  # [DIM, DIM] block-diag


def kernel(x, context, q_w, kv_w, qn_scale, qn_bias, kn_scale, kn_bias,
           proj_w, proj_b, _trace=False):
    global LAST_RUN
    x = np.asarray(x, np.float32)
    context = np.asarray(context, np.float32)
    q_w = np.asarray(q_w, np.float32)
    kv_w = np.asarray(kv_w, np.float32)
    proj_w = np.asarray(proj_w, np.float32)
    proj_b = np.asarray(proj_b, np.float32)
    qn_scale = np.asarray(qn_scale, np.float32)
    qn_bias = np.asarray(qn_bias, np.float32)
    kn_scale = np.asarray(kn_scale, np.float32)
    kn_bias = np.asarray(kn_bias, np.float32)

    ln_affine_q = not (np.all(qn_scale == 1.0) and np.all(qn_bias == 0.0))
    ln_affine_k = not (np.all(kn_scale == 1.0) and np.all(kn_bias == 0.0))

    key = (ln_affine_q, ln_affine_k)
    if key not in _program_cache:
        _program_cache[key] = _build_program(*key)
    nc = _program_cache[key]

    C = _center_mat()
    qw_c = (q_w.astype(np.float64) @ C).astype(np.float32)
    kw_c = (kv_w[:, :DIM].astype(np.float64) @ C).astype(np.float32)
    vw_full = np.ascontiguousarray(kv_w[:, DIM:])

    inv_freq = 1.0 / (10000.0 ** (np.arange(0, HD, 2, dtype=np.float32) / HD))
    ang = np.arange(S, dtype=np.float32)[:, None] * inv_freq
    cos_t = np.cos(ang).astype(np.float32)
    sin_t = np.sin(ang).astype(np.float32)

    in_maps = []
    for core in range(B * G):
        b, g = divmod(core, G)
        sl = slice(g * DL, (g + 1) * DL)
        in_maps.append({
            "x": np.ascontiguousarray(x[b]),
            "ctx": np.ascontiguousarray(context[b]),
            "qw": np.ascontiguousarray(qw_c[:, sl]),
            "kw": np.ascontiguousarray(kw_c[:, sl]),
            "vw": np.ascontiguousarray(vw_full[:, sl]),
            "pw": np.ascontiguousarray(proj_w[sl, :]),
            "cos": cos_t, "sin": sin_t,
            "qs": qn_scale, "qb": qn_bias, "ks": kn_scale, "kb": kn_bias,
        })

    LAST_RUN = run_bass_kernel_spmd(
        nc, in_maps, list(range(B * G)), trace=_trace
    )
    res = LAST_RUN.results

    out = np.zeros((B, S, DIM), np.float32)
    for core in range(B * G):
        out[core // G] += res[core]["y"]
    out += proj_b[None, None, :]
    return out


# revision 15
# speedup vs baseline: 1.8689x; 1.8353x over previous
"""Trainium2 Bass kernel for CrossAttention (B=4, S=S_ctx=2048, D=1024, H=16, Hd=64).

Sharding: 8 cores = batch (4) x head-group (2 groups of 8 heads).
Each core computes, for its (b, g):
    q = x_b @ qw_g          (per-head mean-centering folded into qw on host)
    k = ctx_b @ kw_g        (same)
    v = ctx_b @ vw_g
    per-head LN (rstd only; mean is zero by construction), RoPE on q,
    softmax(q k^T / 8) v per head, partial out-proj with this group's proj_w
    rows.  Host sums the two group partials per batch and adds proj bias.

On-core dataflow (all f32):
  - per 128-token tile: DMA in, 8 PE-transposes -> channel-major tile,
    stationary for the K/V/Q projection matmuls (no full transposed copy)
  - LN-rstd (+ optional affine) and RoPE applied token-major (free-dim math),
    then Q/K PE-transposed into head-dim-major QT/KT for attention
  - logitsT[t, s] per head-pair via two row-tiled matmuls (K=64 each)
  - exp on ScalarE straight out of PSUM (1/sqrt(Hd) folded into activation scale)
  - out^T accumulated with stationary [V_h | ones]; ones row gives softmax denom
  - normalize via DVE reciprocal + DMA partition-broadcast
  - out-proj: attn-outT tiles stationary, proj_w moving, token-major result
"""

import numpy as np
from contextlib import ExitStack

import concourse.bacc as bacc
import concourse.bass as bass
import concourse.tile as tile
from concourse import mybir
from concourse.bass_utils import run_bass_kernel_spmd
from concourse.masks import make_identity

F32 = mybir.dt.float32
F32R = mybir.dt.float32r
AF = mybir.ActivationFunctionType

B, S, DIM = 4, 2048, 1024
H, HD = 16, 64
G = 2                  # head groups (tensor-parallel dim)
HL = H // G            # heads per core = 8
DL = HL * HD           # local head dims = 512
P = 128
NT = S // P            # 16 token tiles
NR = DIM // P          # 8 channel tiles
EPS = 1e-5

_program_cache = {}
LAST_RUN = None        # BassKernelResults of most recent run (for test harness)


def _mk_ap(ap, dims):
    """Raw AP on the same tensor/offset with explicit [step, count] dims."""
    return bass.AP(tensor=ap.tensor, offset=ap.offset, ap=list(dims))


def _build_program(ln_affine_q, ln_affine_k, trace=False):
    nc = bacc.Bacc(None, target_bir_lowering=False, debug=False)

    x_d = nc.dram_tensor("x", [S, DIM], F32, kind="ExternalInput")
    ctx_d = nc.dram_tensor("ctx", [S, DIM], F32, kind="ExternalInput")
    qw_d = nc.dram_tensor("qw", [DIM, DL], F32R, kind="ExternalInput")
    kw_d = nc.dram_tensor("kw", [DIM, DL], F32R, kind="ExternalInput")
    vw_d = nc.dram_tensor("vw", [DIM, DL], F32R, kind="ExternalInput")
    pw_d = nc.dram_tensor("pw", [DL, DIM], F32R, kind="ExternalInput")
    cos_d = nc.dram_tensor("cos", [S, HD // 2], F32, kind="ExternalInput")
    sin_d = nc.dram_tensor("sin", [S, HD // 2], F32, kind="ExternalInput")
    qs_d = nc.dram_tensor("qs", [HD], F32, kind="ExternalInput")
    qb_d = nc.dram_tensor("qb", [HD], F32, kind="ExternalInput")
    ks_d = nc.dram_tensor("ks", [HD], F32, kind="ExternalInput")
    kb_d = nc.dram_tensor("kb", [HD], F32, kind="ExternalInput")
    y_d = nc.dram_tensor("y", [S, DIM], F32, kind="ExternalOutput")
    den_d = nc.dram_tensor("den_scratch", [4, HL, S // 4], F32)  # internal

    with tile.TileContext(nc) as tc, ExitStack() as top:
        const = top.enter_context(tc.tile_pool(name="const", bufs=1))
        ident = const.tile([P, P], F32)
        make_identity(nc, ident[:])

        eps_sb = const.tile([P, 1], F32)
        nc.vector.memset(eps_sb[:], EPS)
        ones_sb = const.tile([P, HD], F32R)
        nc.vector.memset(ones_sb[:].bitcast(F32), 1.0)

        cos_sb = const.tile([P, NT, HD // 2], F32)
        sin_sb = const.tile([P, NT, HD // 2], F32)
        nc.sync.dma_start(cos_sb[:], cos_d[:].rearrange("(i p) f -> p i f", p=P))
        nc.sync.dma_start(sin_sb[:], sin_d[:].rearrange("(i p) f -> p i f", p=P))

        ln_tiles = {}
        for flag, s_t, b_t, key in (
            (ln_affine_q, qs_d, qb_d, "q"),
            (ln_affine_k, ks_d, kb_d, "k"),
        ):
            if flag:
                st = const.tile([P, HD], F32)
                bt = const.tile([P, HD], F32)
                nc.gpsimd.dma_start(st[:], s_t[:].partition_broadcast(P))
                nc.gpsimd.dma_start(bt[:], b_t[:].partition_broadcast(P))
                ln_tiles[key] = (st, bt)

        persist = top.enter_context(tc.tile_pool(name="persist", bufs=1))
        QT = persist.tile([P, HL // 2, S], F32R)         # [d-par, pair, s]
        KT = persist.tile([P, HL // 2, S], F32R)         # [d-par, pair, t]
        Vaug = persist.tile([P, NT, HL, HD + 1], F32R)   # [t-par, t-tile, h, e|1]
        rstdQ = persist.tile([P, NT, HL], F32)
        rstdK = persist.tile([P, NT, HL], F32)
        nc.vector.memset(Vaug[:, :, :, HD : HD + 1].bitcast(F32), 1.0)

        def rstd_of(work, nat, dst):
            """dst[:, :] = 1/sqrt(mean(nat^2 per head) + eps); nat is [P, DL]."""
            sq = work.tile([P, DL], F32, tag="sq")
            nc.vector.tensor_mul(sq[:], nat[:], nat[:])
            sums = work.tile([P, HL], F32, tag="sums")
            nc.vector.tensor_reduce(
                sums[:],
                sq[:].rearrange("p (h d) -> p h d", h=HL),
                axis=mybir.AxisListType.X,
                op=mybir.AluOpType.add,
            )
            sdt = work.tile([P, HL], F32, tag="sdt")
            nc.scalar.activation(
                sdt[:], sums[:], AF.Sqrt, bias=eps_sb[:], scale=1.0 / HD
            )
            nc.vector.reciprocal(dst, sdt[:])

        def apply_affine(nat3, key):
            if key in ln_tiles:
                st, bt = ln_tiles[key]
                stb = _mk_ap(st[:], [st[:].ap[0], [0, HL], [1, HD]])
                btb = _mk_ap(bt[:], [bt[:].ap[0], [0, HL], [1, HD]])
                nc.vector.tensor_mul(nat3, nat3, stb)
                nc.vector.tensor_add(nat3, nat3, btb)

        # ================= projection phases =================
        with ExitStack() as ph:
            tp = ph.enter_context(tc.tile_pool(name="tin", bufs=3))
            xtp = ph.enter_context(tc.tile_pool(name="xt", bufs=2))
            wp = ph.enter_context(tc.tile_pool(name="w", bufs=1))
            work = ph.enter_context(tc.tile_pool(name="work", bufs=2))
            tr_ps = ph.enter_context(tc.tile_pool(name="tr_ps", bufs=2, space="PSUM"))
            mm_ps = ph.enter_context(tc.tile_pool(name="mm_ps", bufs=2, space="PSUM"))
            ot_ps = ph.enter_context(tc.tile_pool(name="ot_ps", bufs=2, space="PSUM"))

            kw_sb = wp.tile([P, NR, DL], F32R)
            vw_sb = wp.tile([P, NR, DL], F32R)
            qw_sb = wp.tile([P, NR, DL], F32R)
            nc.sync.dma_start(kw_sb[:], kw_d[:].rearrange("(r p) d -> p r d", p=P))
            nc.sync.dma_start(vw_sb[:], vw_d[:].rearrange("(r p) d -> p r d", p=P))
            nc.sync.dma_start(qw_sb[:], qw_d[:].rearrange("(r p) d -> p r d", p=P))

            def transposed_tile(inp_dram, t):
                """Load token-tile t and return channel-major [P, NR, P] tile."""
                t_in = tp.tile([P, DIM], F32, tag="t_in")
                nc.sync.dma_start(t_in[:], inp_dram[t * P : (t + 1) * P, :])
                xt = xtp.tile([P, NR, P], F32R, tag="xt")
                for r2 in range(2):
                    ps4 = tr_ps.tile([P, 4 * P], F32, tag="ps4")
                    for j in range(4):
                        nc.tensor.transpose(
                            ps4[:, j * P : (j + 1) * P],
                            t_in[:, (r2 * 4 + j) * P : (r2 * 4 + j + 1) * P],
                            ident[:],
                        )
                    nc.scalar.copy(xt[:, r2 * 4 : (r2 + 1) * 4, :], ps4[:])
                return xt

            def out_transpose(nat, OT, t):
                """PE-transpose token-major [P, DL] into OT[:, :, t*P:...]."""
                for r4 in range(DL // P):
                    psT = ot_ps.tile([P, P], F32, tag="psT")
                    nc.tensor.transpose(
                        psT[:], nat[:, r4 * P : (r4 + 1) * P], ident[:]
                    )
                    nc.scalar.copy(OT[:, r4, t * P : (t + 1) * P], psT[:])

            # context side: K and V
            for t in range(NT):
                xt = transposed_tile(ctx_d, t)
                ps_k = mm_ps.tile([P, DL], F32, tag="ps_k")
                ps_v = mm_ps.tile([P, DL], F32, tag="ps_v")
                for r in range(NR):
                    nc.tensor.matmul(
                        ps_k[:], xt[:, r, :], kw_sb[:, r, :],
                        start=(r == 0), stop=(r == NR - 1),
                    )
                for r in range(NR):
                    nc.tensor.matmul(
                        ps_v[:], xt[:, r, :], vw_sb[:, r, :],
                        start=(r == 0), stop=(r == NR - 1),
                    )
                nc.scalar.copy(
                    Vaug[:, t, :, 0:HD],
                    ps_v[:].rearrange("p (h d) -> p h d", h=HL),
                )
                k_nat = work.tile([P, DL], F32, tag="k_nat")
                nc.scalar.copy(k_nat[:], ps_k[:])
                rstd_of(work, k_nat, rstdK[:, t, :])
                rb = _mk_ap(rstdK[:, t, :], [rstdK[:].ap[0], [1, HL], [0, HD]])
                k3 = k_nat[:].rearrange("p (h d) -> p h d", h=HL)
                nc.vector.tensor_mul(k3, k3, rb)
                apply_affine(k3, "k")
                out_transpose(k_nat, KT, t)

            # x side: Q (+ RoPE)
            for t in range(NT):
                xt = transposed_tile(x_d, t)
                ps_q = mm_ps.tile([P, DL], F32, tag="ps_k")
                for r in range(NR):
                    nc.tensor.matmul(
                        ps_q[:], xt[:, r, :], qw_sb[:, r, :],
                        start=(r == 0), stop=(r == NR - 1),
                    )
                q_nat = work.tile([P, DL], F32, tag="k_nat")
                nc.scalar.copy(q_nat[:], ps_q[:])
                rstd_of(work, q_nat, rstdQ[:, t, :])
                rb = _mk_ap(rstdQ[:, t, :], [rstdQ[:].ap[0], [1, HL], [0, HD]])
                q3 = q_nat[:].rearrange("p (h d) -> p h d", h=HL)
                nc.vector.tensor_mul(q3, q3, rb)
                apply_affine(q3, "q")
                # RoPE: view [p, h, 2, 32]
                qcos = work.tile([P, DL], F32, tag="qcos")
                qsin = work.tile([P, DL], F32, tag="qsin")
                cb = _mk_ap(cos_sb[:, t, :],
                            [cos_sb[:].ap[0], [0, HL], [0, 2], [1, HD // 2]])
                sb = _mk_ap(sin_sb[:, t, :],
                            [sin_sb[:].ap[0], [0, HL], [0, 2], [1, HD // 2]])
                q4 = q_nat[:].rearrange("p (h two f) -> p h two f", h=HL, two=2)
                qcos4 = qcos[:].rearrange("p (h two f) -> p h two f", h=HL, two=2)
                qsin4 = qsin[:].rearrange("p (h two f) -> p h two f", h=HL, two=2)
                nc.vector.tensor_mul(qcos4, q4, cb)
                nc.vector.tensor_mul(qsin4, q4, sb)
                nc.vector.tensor_sub(
                    q4[:, :, 0, :], qcos4[:, :, 0, :], qsin4[:, :, 1, :]
                )
                nc.vector.tensor_add(
                    q4[:, :, 1, :], qsin4[:, :, 0, :], qcos4[:, :, 1, :]
                )
                out_transpose(q_nat, QT, t)

        # ================= attention + out-proj =================
        with ExitStack() as ph:
            pwp = ph.enter_context(tc.tile_pool(name="pw", bufs=1))
            l_ps = ph.enter_context(tc.tile_pool(name="l_ps", bufs=2, space="PSUM"))
            o_ps = ph.enter_context(tc.tile_pool(name="o_ps", bufs=2, space="PSUM"))
            ex_pool = ph.enter_context(tc.tile_pool(name="ex", bufs=3))
            ao_pool = ph.enter_context(tc.tile_pool(name="ao", bufs=2))
            nrm_pool = ph.enter_context(tc.tile_pool(name="nrm", bufs=4))
            y_pool = ph.enter_context(tc.tile_pool(name="y", bufs=3))

            pw_sb = pwp.tile([P, DL // P, DIM], F32R)
            nc.sync.dma_start(pw_sb[:], pw_d[:].rearrange("(r p) n -> p r n", p=P))

            NSC = 4
            SC = S // NSC          # 512

            for sc in range(NSC):
                aoT = ao_pool.tile([P, DL // P, SC], F32R, tag="aoT")
                for r in range(HL // 2):
                    ps_o = o_ps.tile([HD + 1, 2, SC], F32, tag="ps_o")
                    for t in range(NT):
                        ps_l = l_ps.tile([P, 2 * SC], F32, tag="ps_l")
                        nc.tensor.matmul(
                            ps_l[:, 0:SC],
                            KT[0:HD, r, t * P : (t + 1) * P],
                            QT[0:HD, r, sc * SC : (sc + 1) * SC],
                            start=True, stop=True,
                            tile_position=(0, 0),
                        )
                        nc.tensor.matmul(
                            ps_l[:, SC : 2 * SC],
                            KT[HD:P, r, t * P : (t + 1) * P],
                            QT[HD:P, r, sc * SC : (sc + 1) * SC],
                            start=True, stop=True,
                            tile_position=(HD, 0),
                        )
                        ex = ex_pool.tile([P, 2 * SC], F32R, tag="ex")
                        nc.scalar.activation(
                            ex[:], ps_l[:], AF.Exp, scale=1.0 / np.sqrt(HD)
                        )
                        for j in range(2):
                            nc.tensor.matmul(
                                ps_o[:, j, :],
                                Vaug[:, t, 2 * r + j, :],
                                ex[:, j * SC : (j + 1) * SC],
                                start=(t == 0), stop=(t == NT - 1),
                            )
                    for j in range(2):
                        den = nrm_pool.tile([P, SC], F32R, tag="den")
                        with nc.allow_low_precision(reason="f32r recip, 2^-13 rel"):
                            nc.vector.reciprocal(
                                den[HD : HD + 1, :], ps_o[HD : HD + 1, j, :]
                            )
                        # broadcast row 64 -> 64 partitions via K=1 matmul
                        ps_b = l_ps.tile([HD, SC], F32, tag="ps_l")
                        nc.tensor.matmul(
                            ps_b[:],
                            ones_sb[HD : HD + 1, :],
                            den[HD : HD + 1, :],
                            start=True, stop=True,
                            tile_position=(HD, 0),
                        )
                        denB = nrm_pool.tile([HD, SC], F32, tag="denB")
                        nc.vector.tensor_copy(denB[:], ps_b[:])
                        if j == 0:
                            nc.vector.tensor_mul(
                                aoT[0:HD, r, :], ps_o[0:HD, 0, :], denB[:]
                            )
                        else:
                            tmpB = nrm_pool.tile([HD, SC], F32R, tag="tmpB")
                            nc.vector.tensor_mul(tmpB[:], ps_o[0:HD, 1, :], denB[:])
                            nc.gpsimd.dma_start(aoT[HD:P, r, :], tmpB[:])
                for si in range(SC // P):
                    y_sb = y_pool.tile([P, DIM], F32, tag="y_sb")
                    for n2 in range(2):
                        ps_y = o_ps.tile([P, DIM // 2], F32, tag="ps_o")
                        for r in range(DL // P):
                            nc.tensor.matmul(
                                ps_y[:],
                                aoT[:, r, si * P : (si + 1) * P],
                                pw_sb[:, r, n2 * (DIM // 2) : (n2 + 1) * (DIM // 2)],
                                start=(r == 0), stop=(r == DL // P - 1),
                            )
                        nc.vector.tensor_copy(
                            y_sb[:, n2 * (DIM // 2) : (n2 + 1) * (DIM // 2)], ps_y[:]
                        )
                    row0 = sc * SC + si * P
                    nc.sync.dma_start(y_d[row0 : row0 + P, :], y_sb[:])

    nc.compile()
    return nc


def _center_mat():
    m = np.eye(HD, dtype=np.float64) - np.ones((HD, HD), dtype=np.float64) / HD
    return np.kron(np.eye(H, dtype=np.float64), m)  # [DIM, DIM] block-diag


def kernel(x, context, q_w, kv_w, qn_scale, qn_bias, kn_scale, kn_bias,
           proj_w, proj_b, _trace=False):
    global LAST_RUN
    x = np.asarray(x, np.float32)
    context = np.asarray(context, np.float32)
    q_w = np.asarray(q_w, np.float32)
    kv_w = np.asarray(kv_w, np.float32)
    proj_w = np.asarray(proj_w, np.float32)
    proj_b = np.asarray(proj_b, np.float32)
    qn_scale = np.asarray(qn_scale, np.float32)
    qn_bias = np.asarray(qn_bias, np.float32)
    kn_scale = np.asarray(kn_scale, np.float32)
    kn_bias = np.asarray(kn_bias, np.float32)

    ln_affine_q = not (np.all(qn_scale == 1.0) and np.all(qn_bias == 0.0))
    ln_affine_k = not (np.all(kn_scale == 1.0) and np.all(kn_bias == 0.0))

    key = (ln_affine_q, ln_affine_k)
    if key not in _program_cache:
        _program_cache[key] = _build_program(*key)
    nc = _program_cache[key]

    C = _center_mat()
    qw_c = (q_w.astype(np.float64) @ C).astype(np.float32)
    kw_c = (kv_w[:, :DIM].astype(np.float64) @ C).astype(np.float32)
    vw_full = np.ascontiguousarray(kv_w[:, DIM:])

    inv_freq = 1.0 / (10000.0 ** (np.arange(0, HD, 2, dtype=np.float32) / HD))
    ang = np.arange(S, dtype=np.float32)[:, None] * inv_freq
    cos_t = np.cos(ang).astype(np.float32)
    sin_t = np.sin(ang).astype(np.float32)

    in_maps = []
    for core in range(B * G):
        b, g = divmod(core, G)
        sl = slice(g * DL, (g + 1) * DL)
        in_maps.append({
            "x": np.ascontiguousarray(x[b]),
            "ctx": np.ascontiguousarray(context[b]),
            "qw": np.ascontiguousarray(qw_c[:, sl]),
            "kw": np.ascontiguousarray(kw_c[:, sl]),
            "vw": np.ascontiguousarray(vw_full[:, sl]),
            "pw": np.ascontiguousarray(proj_w[sl, :]),
            "cos": cos_t, "sin": sin_t,
            "qs": qn_scale, "qb": qn_bias, "ks": kn_scale, "kb": kn_bias,
        })

    LAST_RUN = run_bass_kernel_spmd(
        nc, in_maps, list(range(B * G)), trace=_trace
    )
    res = LAST_RUN.results

    out = np.zeros((B, S, DIM), np.float32)
    for core in range(B * G):
        out[core // G] += res[core]["y"]
    out += proj_b[None, None, :]
    return out


# revision 17
# speedup vs baseline: 2.0366x; 1.0897x over previous
"""Trainium2 Bass kernel for CrossAttention (B=4, S=S_ctx=2048, D=1024, H=16, Hd=64).

Sharding: 8 cores = batch (4) x head-group (2 groups of 8 heads).
Each core computes, for its (b, g):
    q = x_b @ qw_g          (per-head mean-centering folded into qw on host)
    k = ctx_b @ kw_g        (same)
    v = ctx_b @ vw_g
    per-head LN (rstd only; mean is zero by construction), RoPE on q,
    softmax(q k^T / 8) v per head, partial out-proj with this group's proj_w
    rows.  Host sums the two group partials per batch and adds proj bias.

On-core dataflow (all f32):
  - per 128-token tile: DMA in, 8 PE-transposes -> channel-major tile,
    stationary for the K/V/Q projection matmuls (no full transposed copy)
  - LN-rstd (+ optional affine) and RoPE applied token-major (free-dim math),
    then Q/K PE-transposed into head-dim-major QT/KT for attention
  - logitsT[t, s] per head-pair via two row-tiled matmuls (K=64 each)
  - exp on ScalarE straight out of PSUM (1/sqrt(Hd) folded into activation scale)
  - out^T accumulated with stationary [V_h | ones]; ones row gives softmax denom
  - normalize via DVE reciprocal + DMA partition-broadcast
  - out-proj: attn-outT tiles stationary, proj_w moving, token-major result
"""

import numpy as np
from contextlib import ExitStack

import concourse.bacc as bacc
import concourse.bass as bass
import concourse.tile as tile
from concourse import mybir
from concourse.bass_utils import run_bass_kernel_spmd
from concourse.masks import make_identity

F32 = mybir.dt.float32
F32R = mybir.dt.float32r
AF = mybir.ActivationFunctionType

B, S, DIM = 4, 2048, 1024
H, HD = 16, 64
G = 2                  # head groups (tensor-parallel dim)
HL = H // G            # heads per core = 8
DL = HL * HD           # local head dims = 512
P = 128
NT = S // P            # 16 token tiles
NR = DIM // P          # 8 channel tiles
EPS = 1e-5

_program_cache = {}
LAST_RUN = None        # BassKernelResults of most recent run (for test harness)


def _mk_ap(ap, dims):
    """Raw AP on the same tensor/offset with explicit [step, count] dims."""
    return bass.AP(tensor=ap.tensor, offset=ap.offset, ap=list(dims))


def _build_program(ln_affine_q, ln_affine_k, trace=False):
    nc = bacc.Bacc(None, target_bir_lowering=False, debug=False)

    x_d = nc.dram_tensor("x", [S, DIM], F32, kind="ExternalInput")
    ctx_d = nc.dram_tensor("ctx", [S, DIM], F32, kind="ExternalInput")
    qw_d = nc.dram_tensor("qw", [DIM, DL], F32R, kind="ExternalInput")
    kw_d = nc.dram_tensor("kw", [DIM, DL], F32R, kind="ExternalInput")
    vw_d = nc.dram_tensor("vw", [DIM, DL], F32R, kind="ExternalInput")
    pw_d = nc.dram_tensor("pw", [DL, DIM], F32R, kind="ExternalInput")
    cos_d = nc.dram_tensor("cos", [S, HD // 2], F32, kind="ExternalInput")
    sin_d = nc.dram_tensor("sin", [S, HD // 2], F32, kind="ExternalInput")
    qs_d = nc.dram_tensor("qs", [HD], F32, kind="ExternalInput")
    qb_d = nc.dram_tensor("qb", [HD], F32, kind="ExternalInput")
    ks_d = nc.dram_tensor("ks", [HD], F32, kind="ExternalInput")
    kb_d = nc.dram_tensor("kb", [HD], F32, kind="ExternalInput")
    y_d = nc.dram_tensor("y", [S, DIM], F32, kind="ExternalOutput")
    den_d = nc.dram_tensor("den_scratch", [4, HL, S // 4], F32)  # internal

    with tile.TileContext(nc) as tc, ExitStack() as top:
        const = top.enter_context(tc.tile_pool(name="const", bufs=1))
        ident = const.tile([P, P], F32)
        make_identity(nc, ident[:])

        eps_sb = const.tile([P, 1], F32)
        nc.vector.memset(eps_sb[:], EPS)
        ones_sb = const.tile([P, HD], F32R)
        nc.vector.memset(ones_sb[:].bitcast(F32), 1.0)

        cos_sb = const.tile([P, NT, HD // 2], F32)
        sin_sb = const.tile([P, NT, HD // 2], F32)
        nc.sync.dma_start(cos_sb[:], cos_d[:].rearrange("(i p) f -> p i f", p=P))
        nc.sync.dma_start(sin_sb[:], sin_d[:].rearrange("(i p) f -> p i f", p=P))

        ln_tiles = {}
        for flag, s_t, b_t, key in (
            (ln_affine_q, qs_d, qb_d, "q"),
            (ln_affine_k, ks_d, kb_d, "k"),
        ):
            if flag:
                st = const.tile([P, HD], F32)
                bt = const.tile([P, HD], F32)
                nc.gpsimd.dma_start(st[:], s_t[:].partition_broadcast(P))
                nc.gpsimd.dma_start(bt[:], b_t[:].partition_broadcast(P))
                ln_tiles[key] = (st, bt)

        persist = top.enter_context(tc.tile_pool(name="persist", bufs=1))
        QT = persist.tile([P, HL // 2, S], F32R)         # [d-par, pair, s]
        KT = persist.tile([P, HL // 2, S], F32R)         # [d-par, pair, t]
        Vaug = persist.tile([P, NT, HL, HD + 1], F32R)   # [t-par, t-tile, h, e|1]
        rstdQ = persist.tile([P, NT, HL], F32)
        rstdK = persist.tile([P, NT, HL], F32)
        nc.vector.memset(Vaug[:, :, :, HD : HD + 1].bitcast(F32), 1.0)

        def rstd_of(work, nat, dst):
            """dst[:, :] = 1/sqrt(mean(nat^2 per head) + eps); nat is [P, DL]."""
            sq = work.tile([P, DL], F32, tag="sq")
            nc.vector.tensor_mul(sq[:], nat[:], nat[:])
            sums = work.tile([P, HL], F32, tag="sums")
            nc.vector.tensor_reduce(
                sums[:],
                sq[:].rearrange("p (h d) -> p h d", h=HL),
                axis=mybir.AxisListType.X,
                op=mybir.AluOpType.add,
            )
            sdt = work.tile([P, HL], F32, tag="sdt")
            nc.scalar.activation(
                sdt[:], sums[:], AF.Sqrt, bias=eps_sb[:], scale=1.0 / HD
            )
            nc.vector.reciprocal(dst, sdt[:])

        def apply_affine(nat3, key):
            if key in ln_tiles:
                st, bt = ln_tiles[key]
                stb = _mk_ap(st[:], [st[:].ap[0], [0, HL], [1, HD]])
                btb = _mk_ap(bt[:], [bt[:].ap[0], [0, HL], [1, HD]])
                nc.vector.tensor_mul(nat3, nat3, stb)
                nc.vector.tensor_add(nat3, nat3, btb)

        # ================= projection phases =================
        with ExitStack() as ph:
            tp = ph.enter_context(tc.tile_pool(name="tin", bufs=3))
            xtp = ph.enter_context(tc.tile_pool(name="xt", bufs=2))
            wp = ph.enter_context(tc.tile_pool(name="w", bufs=1))
            work = ph.enter_context(tc.tile_pool(name="work", bufs=2))
            tr_ps = ph.enter_context(tc.tile_pool(name="tr_ps", bufs=2, space="PSUM"))
            mm_ps = ph.enter_context(tc.tile_pool(name="mm_ps", bufs=2, space="PSUM"))
            ot_ps = ph.enter_context(tc.tile_pool(name="ot_ps", bufs=2, space="PSUM"))

            kw_sb = wp.tile([P, NR, DL], F32R)
            vw_sb = wp.tile([P, NR, DL], F32R)
            qw_sb = wp.tile([P, NR, DL], F32R)
            nc.sync.dma_start(kw_sb[:], kw_d[:].rearrange("(r p) d -> p r d", p=P))
            nc.sync.dma_start(vw_sb[:], vw_d[:].rearrange("(r p) d -> p r d", p=P))
            nc.sync.dma_start(qw_sb[:], qw_d[:].rearrange("(r p) d -> p r d", p=P))

            def transposed_tile(inp_dram, t):
                """Load token-tile t and return channel-major [P, NR, P] tile."""
                t_in = tp.tile([P, DIM], F32, tag="t_in")
                nc.sync.dma_start(t_in[:], inp_dram[t * P : (t + 1) * P, :])
                xt = xtp.tile([P, NR, P], F32R, tag="xt")
                for r2 in range(2):
                    ps4 = tr_ps.tile([P, 4 * P], F32, tag="ps4")
                    for j in range(4):
                        nc.tensor.transpose(
                            ps4[:, j * P : (j + 1) * P],
                            t_in[:, (r2 * 4 + j) * P : (r2 * 4 + j + 1) * P],
                            ident[:],
                        )
                    nc.scalar.copy(xt[:, r2 * 4 : (r2 + 1) * 4, :], ps4[:])
                return xt

            def out_transpose(nat, OT, t):
                """PE-transpose token-major [P, DL] into OT[:, :, t*P:...]."""
                for r4 in range(DL // P):
                    psT = ot_ps.tile([P, P], F32, tag="psT")
                    nc.tensor.transpose(
                        psT[:], nat[:, r4 * P : (r4 + 1) * P], ident[:]
                    )
                    nc.scalar.copy(OT[:, r4, t * P : (t + 1) * P], psT[:])

            # context side: K and V
            for t in range(NT):
                xt = transposed_tile(ctx_d, t)
                ps_k = mm_ps.tile([P, DL], F32, tag="ps_k")
                ps_v = mm_ps.tile([P, DL], F32, tag="ps_v")
                for r in range(NR):
                    nc.tensor.matmul(
                        ps_k[:], xt[:, r, :], kw_sb[:, r, :],
                        start=(r == 0), stop=(r == NR - 1),
                    )
                for r in range(NR):
                    nc.tensor.matmul(
                        ps_v[:], xt[:, r, :], vw_sb[:, r, :],
                        start=(r == 0), stop=(r == NR - 1),
                    )
                nc.scalar.copy(
                    Vaug[:, t, :, 0:HD],
                    ps_v[:].rearrange("p (h d) -> p h d", h=HL),
                )
                k_nat = work.tile([P, DL], F32, tag="k_nat")
                nc.scalar.copy(k_nat[:], ps_k[:])
                rstd_of(work, k_nat, rstdK[:, t, :])
                rb = _mk_ap(rstdK[:, t, :], [rstdK[:].ap[0], [1, HL], [0, HD]])
                k3 = k_nat[:].rearrange("p (h d) -> p h d", h=HL)
                nc.vector.tensor_mul(k3, k3, rb)
                apply_affine(k3, "k")
                out_transpose(k_nat, KT, t)

            # x side: Q (+ RoPE)
            for t in range(NT):
                xt = transposed_tile(x_d, t)
                ps_q = mm_ps.tile([P, DL], F32, tag="ps_k")
                for r in range(NR):
                    nc.tensor.matmul(
                        ps_q[:], xt[:, r, :], qw_sb[:, r, :],
                        start=(r == 0), stop=(r == NR - 1),
                    )
                q_nat = work.tile([P, DL], F32, tag="k_nat")
                nc.scalar.copy(q_nat[:], ps_q[:])
                rstd_of(work, q_nat, rstdQ[:, t, :])
                rb = _mk_ap(rstdQ[:, t, :], [rstdQ[:].ap[0], [1, HL], [0, HD]])
                q3 = q_nat[:].rearrange("p (h d) -> p h d", h=HL)
                nc.vector.tensor_mul(q3, q3, rb)
                apply_affine(q3, "q")
                # RoPE: view [p, h, 2, 32]
                qcos = work.tile([P, DL], F32, tag="qcos")
                qsin = work.tile([P, DL], F32, tag="qsin")
                cb = _mk_ap(cos_sb[:, t, :],
                            [cos_sb[:].ap[0], [0, HL], [0, 2], [1, HD // 2]])
                sb = _mk_ap(sin_sb[:, t, :],
                            [sin_sb[:].ap[0], [0, HL], [0, 2], [1, HD // 2]])
                q4 = q_nat[:].rearrange("p (h two f) -> p h two f", h=HL, two=2)
                qcos4 = qcos[:].rearrange("p (h two f) -> p h two f", h=HL, two=2)
                qsin4 = qsin[:].rearrange("p (h two f) -> p h two f", h=HL, two=2)
                nc.vector.tensor_mul(qcos4, q4, cb)
                nc.vector.tensor_mul(qsin4, q4, sb)
                nc.vector.tensor_sub(
                    q4[:, :, 0, :], qcos4[:, :, 0, :], qsin4[:, :, 1, :]
                )
                nc.vector.tensor_add(
                    q4[:, :, 1, :], qsin4[:, :, 0, :], qcos4[:, :, 1, :]
                )
                out_transpose(q_nat, QT, t)

        # ================= attention + out-proj =================
        with ExitStack() as ph:
            pwp = ph.enter_context(tc.tile_pool(name="pw", bufs=1))
            l_ps = ph.enter_context(tc.tile_pool(name="l_ps", bufs=2, space="PSUM"))
            o_ps = ph.enter_context(tc.tile_pool(name="o_ps", bufs=2, space="PSUM"))
            ex_pool = ph.enter_context(tc.tile_pool(name="ex", bufs=3))
            ao_pool = ph.enter_context(tc.tile_pool(name="ao", bufs=2))
            nrm_pool = ph.enter_context(tc.tile_pool(name="nrm", bufs=4))
            y_pool = ph.enter_context(tc.tile_pool(name="y", bufs=3))

            pw_sb = pwp.tile([P, DL // P, DIM], F32R)
            nc.sync.dma_start(pw_sb[:], pw_d[:].rearrange("(r p) n -> p r n", p=P))

            NSC = 4
            SC = S // NSC          # 512

            def normalize(ps_o, aoT, r):
                for j in range(2):
                    den = nrm_pool.tile([P, SC], F32R, tag="den")
                    with nc.allow_low_precision(reason="f32r recip, 2^-13 rel"):
                        nc.vector.reciprocal(
                            den[HD : HD + 1, :], ps_o[HD : HD + 1, j, :]
                        )
                    # broadcast row 64 -> 64 partitions via K=1 matmul
                    ps_b = l_ps.tile([HD, SC], F32, tag="ps_l")
                    nc.tensor.matmul(
                        ps_b[:],
                        ones_sb[HD : HD + 1, :],
                        den[HD : HD + 1, :],
                        start=True, stop=True,
                        tile_position=(HD, 0),
                    )
                    denB = nrm_pool.tile([HD, SC], F32, tag="denB")
                    nc.vector.tensor_copy(denB[:], ps_b[:])
                    if j == 0:
                        nc.vector.tensor_mul(
                            aoT[0:HD, r, :], ps_o[0:HD, 0, :], denB[:]
                        )
                    else:
                        tmpB = nrm_pool.tile([HD, SC], F32R, tag="tmpB")
                        nc.vector.tensor_mul(tmpB[:], ps_o[0:HD, 1, :], denB[:])
                        nc.gpsimd.dma_start(aoT[HD:P, r, :], tmpB[:])

            for sc in range(NSC):
                aoT = ao_pool.tile([P, DL // P, SC], F32R, tag="aoT")
                pending = None
                for r in range(HL // 2):
                    ps_o = o_ps.tile([HD + 1, 2, SC], F32, tag="ps_o")
                    for t in range(NT):
                        ps_l = l_ps.tile([P, 2 * SC], F32, tag="ps_l")
                        nc.tensor.matmul(
                            ps_l[:, 0:SC],
                            KT[0:HD, r, t * P : (t + 1) * P],
                            QT[0:HD, r, sc * SC : (sc + 1) * SC],
                            start=True, stop=True,
                            tile_position=(0, 0),
                        )
                        nc.tensor.matmul(
                            ps_l[:, SC : 2 * SC],
                            KT[HD:P, r, t * P : (t + 1) * P],
                            QT[HD:P, r, sc * SC : (sc + 1) * SC],
                            start=True, stop=True,
                            tile_position=(HD, 0),
                        )
                        ex = ex_pool.tile([P, 2 * SC], F32R, tag="ex")
                        nc.scalar.activation(
                            ex[:], ps_l[:], AF.Exp, scale=1.0 / np.sqrt(HD)
                        )
                        for j in range(2):
                            nc.tensor.matmul(
                                ps_o[:, j, :],
                                Vaug[:, t, 2 * r + j, :],
                                ex[:, j * SC : (j + 1) * SC],
                                start=(t == 0), stop=(t == NT - 1),
                            )
                        if t == 3 and pending is not None:
                            normalize(pending, aoT, r - 1)
                            pending = None
                    if r < HL // 2 - 1:
                        pending = ps_o
                    else:
                        normalize(ps_o, aoT, r)
                for si in range(SC // P):
                    y_sb = y_pool.tile([P, DIM], F32, tag="y_sb")
                    for n2 in range(2):
                        ps_y = o_ps.tile([P, DIM // 2], F32, tag="ps_o")
                        for r in range(DL // P):
                            nc.tensor.matmul(
                                ps_y[:],
                                aoT[:, r, si * P : (si + 1) * P],
                                pw_sb[:, r, n2 * (DIM // 2) : (n2 + 1) * (DIM // 2)],
                                start=(r == 0), stop=(r == DL // P - 1),
                            )
                        nc.vector.tensor_copy(
                            y_sb[:, n2 * (DIM // 2) : (n2 + 1) * (DIM // 2)], ps_y[:]
                        )
                    row0 = sc * SC + si * P
                    nc.sync.dma_start(y_d[row0 : row0 + P, :], y_sb[:])

    nc.compile()
    return nc


def _center_mat():
    m = np.eye(HD, dtype=np.float64) - np.ones((HD, HD), dtype=np.float64) / HD
    return np.kron(np.eye(H, dtype=np.float64), m)  # [DIM, DIM] block-diag


def kernel(x, context, q_w, kv_w, qn_scale, qn_bias, kn_scale, kn_bias,
           proj_w, proj_b, _trace=False):
    global LAST_RUN
    x = np.asarray(x, np.float32)
    context = np.asarray(context, np.float32)
    q_w = np.asarray(q_w, np.float32)
    kv_w = np.asarray(kv_w, np.float32)
    proj_w = np.asarray(proj_w, np.float32)
    proj_b = np.asarray(proj_b, np.float32)
    qn_scale = np.asarray(qn_scale, np.float32)
    qn_bias = np.asarray(qn_bias, np.float32)
    kn_scale = np.asarray(kn_scale, np.float32)
    kn_bias = np.asarray(kn_bias, np.float32)

    ln_affine_q = not (np.all(qn_scale == 1.0) and np.all(qn_bias == 0.0))
    ln_affine_k = not (np.all(kn_scale == 1.0) and np.all(kn_bias == 0.0))

    key = (ln_affine_q, ln_affine_k)
    if key not in _program_cache:
        _program_cache[key] = _build_program(*key)
    nc = _program_cache[key]

    C = _center_mat()
    qw_c = (q_w.astype(np.float64) @ C).astype(np.float32)
    kw_c = (kv_w[:, :DIM].astype(np.float64) @ C).astype(np.float32)
    vw_full = np.ascontiguousarray(kv_w[:, DIM:])

    inv_freq = 1.0 / (10000.0 ** (np.arange(0, HD, 2, dtype=np.float32) / HD))
    ang = np.arange(S, dtype=np.float32)[:, None] * inv_freq
    cos_t = np.cos(ang).astype(np.float32)
    sin_t = np.sin(ang).astype(np.float32)

    in_maps = []
    for core in range(B * G):
        b, g = divmod(core, G)
        sl = slice(g * DL, (g + 1) * DL)
        in_maps.append({
            "x": np.ascontiguousarray(x[b]),
            "ctx": np.ascontiguousarray(context[b]),
            "qw": np.ascontiguousarray(qw_c[:, sl]),
            "kw": np.ascontiguousarray(kw_c[:, sl]),
            "vw": np.ascontiguousarray(vw_full[:, sl]),
            "pw": np.ascontiguousarray(proj_w[sl, :]),
            "cos": cos_t, "sin": sin_t,
            "qs": qn_scale, "qb": qn_bias, "ks": kn_scale, "kb": kn_bias,
        })

    LAST_RUN = run_bass_kernel_spmd(
        nc, in_maps, list(range(B * G)), trace=_trace
    )
    res = LAST_RUN.results

    out = np.zeros((B, S, DIM), np.float32)
    for core in range(B * G):
        out[core // G] += res[core]["y"]
    out += proj_b[None, None, :]
    return out


# revision 18
# speedup vs baseline: 2.2880x; 1.1234x over previous
"""Trainium2 Bass kernel for CrossAttention (B=4, S=S_ctx=2048, D=1024, H=16, Hd=64).

Sharding: 8 cores = batch (4) x head-group (2 groups of 8 heads).
Each core computes, for its (b, g):
    q = x_b @ qw_g          (per-head mean-centering folded into qw on host)
    k = ctx_b @ kw_g        (same)
    v = ctx_b @ vw_g
    per-head LN (rstd only; mean is zero by construction), RoPE on q,
    softmax(q k^T / 8) v per head, partial out-proj with this group's proj_w
    rows.  Host sums the two group partials per batch and adds proj bias.

On-core dataflow (all f32):
  - per 128-token tile: DMA in, 8 PE-transposes -> channel-major tile,
    stationary for the K/V/Q projection matmuls (no full transposed copy)
  - LN-rstd (+ optional affine) and RoPE applied token-major (free-dim math),
    then Q/K PE-transposed into head-dim-major QT/KT for attention
  - logitsT[t, s] per head-pair via two row-tiled matmuls (K=64 each)
  - exp on ScalarE straight out of PSUM (1/sqrt(Hd) folded into activation scale)
  - out^T accumulated with stationary [V_h | ones]; ones row gives softmax denom
  - normalize via DVE reciprocal + DMA partition-broadcast
  - out-proj: attn-outT tiles stationary, proj_w moving, token-major result
"""

import numpy as np
from contextlib import ExitStack

import concourse.bacc as bacc
import concourse.bass as bass
import concourse.tile as tile
from concourse import mybir
from concourse.bass_utils import run_bass_kernel_spmd
from concourse.masks import make_identity

F32 = mybir.dt.float32
F32R = mybir.dt.float32r
BF16 = mybir.dt.bfloat16
AF = mybir.ActivationFunctionType

B, S, DIM = 4, 2048, 1024
H, HD = 16, 64
G = 2                  # head groups (tensor-parallel dim)
HL = H // G            # heads per core = 8
DL = HL * HD           # local head dims = 512
P = 128
NT = S // P            # 16 token tiles
NR = DIM // P          # 8 channel tiles
EPS = 1e-5

_program_cache = {}
LAST_RUN = None        # BassKernelResults of most recent run (for test harness)


def _mk_ap(ap, dims):
    """Raw AP on the same tensor/offset with explicit [step, count] dims."""
    return bass.AP(tensor=ap.tensor, offset=ap.offset, ap=list(dims))


def _build_program(ln_affine_q, ln_affine_k, trace=False):
    nc = bacc.Bacc(None, target_bir_lowering=False, debug=False)

    x_d = nc.dram_tensor("x", [S, DIM], F32, kind="ExternalInput")
    ctx_d = nc.dram_tensor("ctx", [S, DIM], F32, kind="ExternalInput")
    qw_d = nc.dram_tensor("qw", [DIM, DL], F32R, kind="ExternalInput")
    kw_d = nc.dram_tensor("kw", [DIM, DL], F32R, kind="ExternalInput")
    vw_d = nc.dram_tensor("vw", [DIM, DL], F32R, kind="ExternalInput")
    pw_d = nc.dram_tensor("pw", [DL, DIM], BF16, kind="ExternalInput")
    cos_d = nc.dram_tensor("cos", [S, HD // 2], F32, kind="ExternalInput")
    sin_d = nc.dram_tensor("sin", [S, HD // 2], F32, kind="ExternalInput")
    qs_d = nc.dram_tensor("qs", [HD], F32, kind="ExternalInput")
    qb_d = nc.dram_tensor("qb", [HD], F32, kind="ExternalInput")
    ks_d = nc.dram_tensor("ks", [HD], F32, kind="ExternalInput")
    kb_d = nc.dram_tensor("kb", [HD], F32, kind="ExternalInput")
    y_d = nc.dram_tensor("y", [S, DIM], F32, kind="ExternalOutput")
    den_d = nc.dram_tensor("den_scratch", [4, HL, S // 4], F32)  # internal

    with tile.TileContext(nc) as tc, ExitStack() as top:
        const = top.enter_context(tc.tile_pool(name="const", bufs=1))
        ident = const.tile([P, P], F32)
        make_identity(nc, ident[:])

        eps_sb = const.tile([P, 1], F32)
        nc.vector.memset(eps_sb[:], EPS)
        ones_sb = const.tile([P, HD], F32R)
        nc.vector.memset(ones_sb[:].bitcast(F32), 1.0)

        cos_sb = const.tile([P, NT, HD // 2], F32)
        sin_sb = const.tile([P, NT, HD // 2], F32)
        nc.sync.dma_start(cos_sb[:], cos_d[:].rearrange("(i p) f -> p i f", p=P))
        nc.sync.dma_start(sin_sb[:], sin_d[:].rearrange("(i p) f -> p i f", p=P))

        ln_tiles = {}
        for flag, s_t, b_t, key in (
            (ln_affine_q, qs_d, qb_d, "q"),
            (ln_affine_k, ks_d, kb_d, "k"),
        ):
            if flag:
                st = const.tile([P, HD], F32)
                bt = const.tile([P, HD], F32)
                nc.gpsimd.dma_start(st[:], s_t[:].partition_broadcast(P))
                nc.gpsimd.dma_start(bt[:], b_t[:].partition_broadcast(P))
                ln_tiles[key] = (st, bt)

        persist = top.enter_context(tc.tile_pool(name="persist", bufs=1))
        QT = persist.tile([P, HL // 2, S], BF16)         # [d-par, pair, s]
        KT = persist.tile([P, HL // 2, S], BF16)         # [d-par, pair, t]
        Vaug = persist.tile([P, NT, HL, HD + 1], BF16)   # [t-par, t-tile, h, e|1]
        rstdQ = persist.tile([P, NT, HL], F32)
        rstdK = persist.tile([P, NT, HL], F32)
        nc.vector.memset(Vaug[:, :, :, HD : HD + 1], 1.0)

        def rstd_of(work, nat, dst):
            """dst[:, :] = 1/sqrt(mean(nat^2 per head) + eps); nat is [P, DL]."""
            sq = work.tile([P, DL], F32, tag="sq")
            nc.vector.tensor_mul(sq[:], nat[:], nat[:])
            sums = work.tile([P, HL], F32, tag="sums")
            nc.vector.tensor_reduce(
                sums[:],
                sq[:].rearrange("p (h d) -> p h d", h=HL),
                axis=mybir.AxisListType.X,
                op=mybir.AluOpType.add,
            )
            sdt = work.tile([P, HL], F32, tag="sdt")
            nc.scalar.activation(
                sdt[:], sums[:], AF.Sqrt, bias=eps_sb[:], scale=1.0 / HD
            )
            nc.vector.reciprocal(dst, sdt[:])

        def apply_affine(nat3, key):
            if key in ln_tiles:
                st, bt = ln_tiles[key]
                stb = _mk_ap(st[:], [st[:].ap[0], [0, HL], [1, HD]])
                btb = _mk_ap(bt[:], [bt[:].ap[0], [0, HL], [1, HD]])
                nc.vector.tensor_mul(nat3, nat3, stb)
                nc.vector.tensor_add(nat3, nat3, btb)

        # ================= projection phases =================
        with ExitStack() as ph:
            tp = ph.enter_context(tc.tile_pool(name="tin", bufs=3))
            xtp = ph.enter_context(tc.tile_pool(name="xt", bufs=2))
            wp = ph.enter_context(tc.tile_pool(name="w", bufs=1))
            work = ph.enter_context(tc.tile_pool(name="work", bufs=2))
            tr_ps = ph.enter_context(tc.tile_pool(name="tr_ps", bufs=2, space="PSUM"))
            mm_ps = ph.enter_context(tc.tile_pool(name="mm_ps", bufs=2, space="PSUM"))
            ot_ps = ph.enter_context(tc.tile_pool(name="ot_ps", bufs=2, space="PSUM"))

            kw_sb = wp.tile([P, NR, DL], F32R)
            vw_sb = wp.tile([P, NR, DL], F32R)
            qw_sb = wp.tile([P, NR, DL], F32R)
            nc.sync.dma_start(kw_sb[:], kw_d[:].rearrange("(r p) d -> p r d", p=P))
            nc.sync.dma_start(vw_sb[:], vw_d[:].rearrange("(r p) d -> p r d", p=P))
            nc.sync.dma_start(qw_sb[:], qw_d[:].rearrange("(r p) d -> p r d", p=P))

            def transposed_tile(inp_dram, t):
                """Load token-tile t and return channel-major [P, NR, P] tile."""
                t_in = tp.tile([P, DIM], F32, tag="t_in")
                nc.sync.dma_start(t_in[:], inp_dram[t * P : (t + 1) * P, :])
                xt = xtp.tile([P, NR, P], F32R, tag="xt")
                for r2 in range(2):
                    ps4 = tr_ps.tile([P, 4 * P], F32, tag="ps4")
                    for j in range(4):
                        nc.tensor.transpose(
                            ps4[:, j * P : (j + 1) * P],
                            t_in[:, (r2 * 4 + j) * P : (r2 * 4 + j + 1) * P],
                            ident[:],
                        )
                    nc.scalar.copy(xt[:, r2 * 4 : (r2 + 1) * 4, :], ps4[:])
                return xt

            def out_transpose(nat, OT, t):
                """PE-transpose token-major [P, DL] into OT[:, :, t*P:...]."""
                for r4 in range(DL // P):
                    psT = ot_ps.tile([P, P], F32, tag="psT")
                    nc.tensor.transpose(
                        psT[:], nat[:, r4 * P : (r4 + 1) * P], ident[:]
                    )
                    nc.scalar.copy(OT[:, r4, t * P : (t + 1) * P], psT[:])

            # context side: K and V
            for t in range(NT):
                xt = transposed_tile(ctx_d, t)
                ps_k = mm_ps.tile([P, DL], F32, tag="ps_k")
                ps_v = mm_ps.tile([P, DL], F32, tag="ps_v")
                for r in range(NR):
                    nc.tensor.matmul(
                        ps_k[:], xt[:, r, :], kw_sb[:, r, :],
                        start=(r == 0), stop=(r == NR - 1),
                    )
                for r in range(NR):
                    nc.tensor.matmul(
                        ps_v[:], xt[:, r, :], vw_sb[:, r, :],
                        start=(r == 0), stop=(r == NR - 1),
                    )
                nc.scalar.copy(
                    Vaug[:, t, :, 0:HD],
                    ps_v[:].rearrange("p (h d) -> p h d", h=HL),
                )
                k_nat = work.tile([P, DL], F32, tag="k_nat")
                nc.scalar.copy(k_nat[:], ps_k[:])
                rstd_of(work, k_nat, rstdK[:, t, :])
                rb = _mk_ap(rstdK[:, t, :], [rstdK[:].ap[0], [1, HL], [0, HD]])
                k3 = k_nat[:].rearrange("p (h d) -> p h d", h=HL)
                nc.vector.tensor_mul(k3, k3, rb)
                apply_affine(k3, "k")
                out_transpose(k_nat, KT, t)

            # x side: Q (+ RoPE)
            for t in range(NT):
                xt = transposed_tile(x_d, t)
                ps_q = mm_ps.tile([P, DL], F32, tag="ps_k")
                for r in range(NR):
                    nc.tensor.matmul(
                        ps_q[:], xt[:, r, :], qw_sb[:, r, :],
                        start=(r == 0), stop=(r == NR - 1),
                    )
                q_nat = work.tile([P, DL], F32, tag="k_nat")
                nc.scalar.copy(q_nat[:], ps_q[:])
                rstd_of(work, q_nat, rstdQ[:, t, :])
                rb = _mk_ap(rstdQ[:, t, :], [rstdQ[:].ap[0], [1, HL], [0, HD]])
                q3 = q_nat[:].rearrange("p (h d) -> p h d", h=HL)
                nc.vector.tensor_mul(q3, q3, rb)
                apply_affine(q3, "q")
                # RoPE: view [p, h, 2, 32]
                qcos = work.tile([P, DL], F32, tag="qcos")
                qsin = work.tile([P, DL], F32, tag="qsin")
                cb = _mk_ap(cos_sb[:, t, :],
                            [cos_sb[:].ap[0], [0, HL], [0, 2], [1, HD // 2]])
                sb = _mk_ap(sin_sb[:, t, :],
                            [sin_sb[:].ap[0], [0, HL], [0, 2], [1, HD // 2]])
                q4 = q_nat[:].rearrange("p (h two f) -> p h two f", h=HL, two=2)
                qcos4 = qcos[:].rearrange("p (h two f) -> p h two f", h=HL, two=2)
                qsin4 = qsin[:].rearrange("p (h two f) -> p h two f", h=HL, two=2)
                nc.vector.tensor_mul(qcos4, q4, cb)
                nc.vector.tensor_mul(qsin4, q4, sb)
                nc.vector.tensor_sub(
                    q4[:, :, 0, :], qcos4[:, :, 0, :], qsin4[:, :, 1, :]
                )
                nc.vector.tensor_add(
                    q4[:, :, 1, :], qsin4[:, :, 0, :], qcos4[:, :, 1, :]
                )
                out_transpose(q_nat, QT, t)

        # ================= attention + out-proj =================
        with ExitStack() as ph:
            pwp = ph.enter_context(tc.tile_pool(name="pw", bufs=1))
            l_ps = ph.enter_context(tc.tile_pool(name="l_ps", bufs=2, space="PSUM"))
            o_ps = ph.enter_context(tc.tile_pool(name="o_ps", bufs=2, space="PSUM"))
            ex_pool = ph.enter_context(tc.tile_pool(name="ex", bufs=3))
            ao_pool = ph.enter_context(tc.tile_pool(name="ao", bufs=2))
            nrm_pool = ph.enter_context(tc.tile_pool(name="nrm", bufs=4))
            y_pool = ph.enter_context(tc.tile_pool(name="y", bufs=3))

            pw_sb = pwp.tile([P, DL // P, DIM], BF16)
            nc.sync.dma_start(pw_sb[:], pw_d[:].rearrange("(r p) n -> p r n", p=P))

            NSC = 4
            SC = S // NSC          # 512

            def normalize(ps_o, aoT, r):
                for j in range(2):
                    den = nrm_pool.tile([P, SC], F32R, tag="den")
                    with nc.allow_low_precision(reason="f32r recip, 2^-13 rel"):
                        nc.vector.reciprocal(
                            den[HD : HD + 1, :], ps_o[HD : HD + 1, j, :]
                        )
                    # broadcast row 64 -> 64 partitions via K=1 matmul
                    ps_b = l_ps.tile([HD, SC], F32, tag="ps_l")
                    nc.tensor.matmul(
                        ps_b[:],
                        ones_sb[HD : HD + 1, :],
                        den[HD : HD + 1, :],
                        start=True, stop=True,
                        tile_position=(HD, 0),
                    )
                    denB = nrm_pool.tile([HD, SC], F32, tag="denB")
                    nc.vector.tensor_copy(denB[:], ps_b[:])
                    if j == 0:
                        nc.vector.tensor_mul(
                            aoT[0:HD, r, :], ps_o[0:HD, 0, :], denB[:]
                        )
                    else:
                        tmpB = nrm_pool.tile([HD, SC], BF16, tag="tmpB")
                        nc.vector.tensor_mul(tmpB[:], ps_o[0:HD, 1, :], denB[:])
                        nc.gpsimd.dma_start(aoT[HD:P, r, :], tmpB[:])

            for sc in range(NSC):
                aoT = ao_pool.tile([P, DL // P, SC], BF16, tag="aoT")
                pending = None
                for r in range(HL // 2):
                    ps_o = o_ps.tile([HD + 1, 2, SC], F32, tag="ps_o")
                    for t in range(NT):
                        ps_l = l_ps.tile([P, 2 * SC], F32, tag="ps_l")
                        nc.tensor.matmul(
                            ps_l[:, 0:SC],
                            KT[0:HD, r, t * P : (t + 1) * P],
                            QT[0:HD, r, sc * SC : (sc + 1) * SC],
                            start=True, stop=True,
                            tile_position=(0, 0),
                        )
                        nc.tensor.matmul(
                            ps_l[:, SC : 2 * SC],
                            KT[HD:P, r, t * P : (t + 1) * P],
                            QT[HD:P, r, sc * SC : (sc + 1) * SC],
                            start=True, stop=True,
                            tile_position=(HD, 0),
                        )
                        ex = ex_pool.tile([P, 2 * SC], BF16, tag="ex")
                        nc.scalar.activation(
                            ex[:], ps_l[:], AF.Exp, scale=1.0 / np.sqrt(HD)
                        )
                        for j in range(2):
                            nc.tensor.matmul(
                                ps_o[:, j, :],
                                Vaug[:, t, 2 * r + j, :],
                                ex[:, j * SC : (j + 1) * SC],
                                start=(t == 0), stop=(t == NT - 1),
                            )
                        if t == 3 and pending is not None:
                            normalize(pending, aoT, r - 1)
                            pending = None
                    if r < HL // 2 - 1:
                        pending = ps_o
                    else:
                        normalize(ps_o, aoT, r)
                for si in range(SC // P):
                    y_sb = y_pool.tile([P, DIM], F32, tag="y_sb")
                    for n2 in range(2):
                        ps_y = o_ps.tile([P, DIM // 2], F32, tag="ps_o")
                        for r in range(DL // P):
                            nc.tensor.matmul(
                                ps_y[:],
                                aoT[:, r, si * P : (si + 1) * P],
                                pw_sb[:, r, n2 * (DIM // 2) : (n2 + 1) * (DIM // 2)],
                                start=(r == 0), stop=(r == DL // P - 1),
                            )
                        nc.vector.tensor_copy(
                            y_sb[:, n2 * (DIM // 2) : (n2 + 1) * (DIM // 2)], ps_y[:]
                        )
                    row0 = sc * SC + si * P
                    nc.sync.dma_start(y_d[row0 : row0 + P, :], y_sb[:])

    nc.compile()
    return nc


def _center_mat():
    m = np.eye(HD, dtype=np.float64) - np.ones((HD, HD), dtype=np.float64) / HD
    return np.kron(np.eye(H, dtype=np.float64), m)  # [DIM, DIM] block-diag


def kernel(x, context, q_w, kv_w, qn_scale, qn_bias, kn_scale, kn_bias,
           proj_w, proj_b, _trace=False):
    global LAST_RUN
    x = np.asarray(x, np.float32)
    context = np.asarray(context, np.float32)
    q_w = np.asarray(q_w, np.float32)
    kv_w = np.asarray(kv_w, np.float32)
    proj_w = np.asarray(proj_w, np.float32)
    proj_b = np.asarray(proj_b, np.float32)
    qn_scale = np.asarray(qn_scale, np.float32)
    qn_bias = np.asarray(qn_bias, np.float32)
    kn_scale = np.asarray(kn_scale, np.float32)
    kn_bias = np.asarray(kn_bias, np.float32)

    ln_affine_q = not (np.all(qn_scale == 1.0) and np.all(qn_bias == 0.0))
    ln_affine_k = not (np.all(kn_scale == 1.0) and np.all(kn_bias == 0.0))

    key = (ln_affine_q, ln_affine_k)
    if key not in _program_cache:
        _program_cache[key] = _build_program(*key)
    nc = _program_cache[key]

    C = _center_mat()
    qw_c = (q_w.astype(np.float64) @ C).astype(np.float32)
    kw_c = (kv_w[:, :DIM].astype(np.float64) @ C).astype(np.float32)
    vw_full = np.ascontiguousarray(kv_w[:, DIM:])

    inv_freq = 1.0 / (10000.0 ** (np.arange(0, HD, 2, dtype=np.float32) / HD))
    ang = np.arange(S, dtype=np.float32)[:, None] * inv_freq
    cos_t = np.cos(ang).astype(np.float32)
    sin_t = np.sin(ang).astype(np.float32)

    in_maps = []
    for core in range(B * G):
        b, g = divmod(core, G)
        sl = slice(g * DL, (g + 1) * DL)
        in_maps.append({
            "x": np.ascontiguousarray(x[b]),
            "ctx": np.ascontiguousarray(context[b]),
            "qw": np.ascontiguousarray(qw_c[:, sl]),
            "kw": np.ascontiguousarray(kw_c[:, sl]),
            "vw": np.ascontiguousarray(vw_full[:, sl]),
            "pw": np.ascontiguousarray(proj_w[sl, :]).astype(mybir.dt.np(BF16)),
            "cos": cos_t, "sin": sin_t,
            "qs": qn_scale, "qb": qn_bias, "ks": kn_scale, "kb": kn_bias,
        })

    LAST_RUN = run_bass_kernel_spmd(
        nc, in_maps, list(range(B * G)), trace=_trace
    )
    res = LAST_RUN.results

    out = np.zeros((B, S, DIM), np.float32)
    for core in range(B * G):
        out[core // G] += res[core]["y"]
    out += proj_b[None, None, :]
    return out
